# Initial kernel scaffold
#
"""Trainium2 Bass kernel for a 6-layer transformer decoder (post-norm, ViT-style).

Sharding: data-parallel over batch — 8 cores x 1 batch element, no collectives.
All activations are kept TRANSPOSED in SBUF ([feature, token]) so every linear
layer is a plain lhsT.T @ rhs matmul with no transposes anywhere.

Softmax is computed over the PARTITION dim (scores stored [sk, sq]); no max
subtraction is needed (scores are O(1) after layernorm), and the denominator
comes for free from a ones-column appended to V in the attn@V matmul.
"""

import numpy as np
import ml_dtypes

BF16 = ml_dtypes.bfloat16

# ---------------- model config (hardcoded; matches the reference) -------------
class Cfg:
    def __init__(self, B=8, S=1024, D=768, H=12, L=6, DFF=2048, n_cores=8):
        self.B, self.S, self.D, self.H, self.L, self.DFF = B, S, D, H, L, DFF
        self.n_cores = n_cores
        self.P = 128
        self.DH = 64                      # head dim (fixed by reference)
        assert D % self.P == 0 and D // H == self.DH
        self.C = D // self.P              # d-chunks
        self.S2 = min(512, S)             # sq tile (matmul N)
        assert S % self.S2 == 0
        self.NJ = S // self.S2            # sq tiles
        self.SK = S // self.P             # sk chunks
        self.FC = DFF // self.P           # ffn chunks
        assert self.FC % 2 == 0
        self.R = self.S2 // self.P        # diag-mask variants
        self.EPS = 1e-5


FULL = Cfg()


def _sinusoidal_pe(max_len, d):
    pos = np.arange(max_len)[:, None].astype(np.float32)
    div = np.exp(np.arange(0, d, 2).astype(np.float32) * (-np.log(10000.0) / d))
    pe = np.zeros((max_len, d), dtype=np.float32)
    pe[:, 0::2] = np.sin(pos * div)
    pe[:, 1::2] = np.cos(pos * div)
    return pe


# ---------------- bass module builder ----------------------------------------
def build_nc(cfg):
    import concourse.bass as bass
    import concourse.mybir as mybir
    import concourse.tile as tile

    P, C, S, S2, NJ, SK, FC, H, DH, L, D, DFF = (
        cfg.P, cfg.C, cfg.S, cfg.S2, cfg.NJ, cfg.SK, cfg.FC, cfg.H, cfg.DH,
        cfg.L, cfg.D, cfg.DFF)
    R = cfg.R
    f32 = mybir.dt.float32
    f32r = mybir.dt.float32r
    bf16 = mybir.dt.bfloat16
    Ident = mybir.ActivationFunctionType.Identity
    Exp = mybir.ActivationFunctionType.Exp
    Relu = mybir.ActivationFunctionType.Relu
    Sqrt = mybir.ActivationFunctionType.Sqrt
    Square = mybir.ActivationFunctionType.Square
    mult = mybir.AluOpType.mult
    addop = mybir.AluOpType.add

    NB = 13 * C + FC
    # bias-pack column offsets
    OB_Q, OB_K, OB_O = 0, C, 2 * C
    OB_CQ, OB_CK, OB_CO = 3 * C, 4 * C, 5 * C
    OB_B1 = 6 * C
    OB_B2 = 6 * C + FC
    OB_LNW = 7 * C + FC
    OB_LNB = 10 * C + FC

    nc = bass.Bass(trn_type="TRN2", name="decoder")

    # ---- DRAM I/O (per-core shapes) ----
    dx32 = nc.dram_tensor("x0t32", [D, S], f32, kind="ExternalInput")
    dxbf = nc.dram_tensor("x0tbf", [D, S], bf16, kind="ExternalInput")
    dmem = nc.dram_tensor("memtbf", [D, S], bf16, kind="ExternalInput")
    dwq_s = nc.dram_tensor("wq_s", [L, D, D], bf16, kind="ExternalInput")
    dwk_s = nc.dram_tensor("wk_s", [L, D, D], bf16, kind="ExternalInput")
    dwv_s = nc.dram_tensor("wv_s", [L, D, D], bf16, kind="ExternalInput")
    dwo_s = nc.dram_tensor("wo_s", [L, D, D], bf16, kind="ExternalInput")
    dwq_c = nc.dram_tensor("wq_c", [L, D, D], bf16, kind="ExternalInput")
    dwk_c = nc.dram_tensor("wk_c", [L, D, D], bf16, kind="ExternalInput")
    dwv_c = nc.dram_tensor("wv_c", [L, D, D], bf16, kind="ExternalInput")
    dwo_c = nc.dram_tensor("wo_c", [L, D, D], bf16, kind="ExternalInput")
    dw1 = nc.dram_tensor("w1", [L, D, DFF], bf16, kind="ExternalInput")
    dw2 = nc.dram_tensor("w2", [L, DFF, D], bf16, kind="ExternalInput")
    dwp = nc.dram_tensor("wp", [D, D], bf16, kind="ExternalInput")
    dbias = nc.dram_tensor("biasp", [L, P, NB], f32, kind="ExternalInput")
    dbvf = nc.dram_tensor("bvf", [L, 2, D], bf16, kind="ExternalInput")
    dbp = nc.dram_tensor("bp6", [P, C], f32, kind="ExternalInput")
    dmask = nc.dram_tensor("masks", [R, P, S2], bf16, kind="ExternalInput")
    dout = nc.dram_tensor("outt", [D, S], f32, kind="ExternalOutput")

    dx32v = dx32.rearrange("(c p) s -> p c s", p=P)
    dxbfv = dxbf.rearrange("(c p) s -> p c s", p=P)
    dmemv = dmem.rearrange("(c p) s -> p c s", p=P)
    doutv = dout.rearrange("(c p) s -> p c s", p=P)

    with tile.TileContext(nc) as tc:
        with (
            tc.tile_pool(name="singles", bufs=1) as singles,
            tc.tile_pool(name="wa", bufs=3) as wa,
            tc.tile_pool(name="wf", bufs=2) as wf,
            tc.tile_pool(name="bp", bufs=2) as bpool,
            tc.tile_pool(name="kv", bufs=2) as kvp,
            tc.tile_pool(name="qp", bufs=1) as qp,
            tc.tile_pool(name="cp", bufs=1) as cp,
            tc.tile_pool(name="pp", bufs=2) as pp,
            tc.tile_pool(name="hp", bufs=1) as hp,
            tc.tile_pool(name="sp", bufs=1) as sp,
            tc.tile_pool(name="st", bufs=2) as st,
            tc.tile_pool(name="pacc", bufs=2, space="PSUM") as pacc,
            tc.tile_pool(name="pfast", bufs=3, space="PSUM") as pfast,
            tc.tile_pool(name="pav", bufs=2, space="PSUM") as pavp,
        ):
            # ---- static tiles ----
            x32 = singles.tile([P, C, S], f32, tag="x32")
            xbf = singles.tile([P, C, S], bf16, tag="xbf")
            memt = singles.tile([P, C, S], bf16, tag="memt")
            masks = singles.tile([P, R, S2], bf16, tag="masks")
            ones2d = singles.tile([P, P], f32, tag="ones2d")
            ones1 = singles.tile([1, DH], f32, tag="ones1")
            epst = singles.tile([P, 1], f32, tag="epst")
            bp_sb = singles.tile([P, C], f32, tag="bp_sb")

            nc.sync.dma_start(out=x32, in_=dx32v)
            nc.sync.dma_start(out=xbf, in_=dxbfv)
            nc.sync.dma_start(out=memt, in_=dmemv)
            nc.sync.dma_start(out=masks, in_=dmask.rearrange("r p s -> p r s"))
            nc.sync.dma_start(out=bp_sb, in_=dbp)
            nc.vector.memset(ones2d, 1.0)
            nc.vector.memset(ones1, 1.0)
            nc.vector.memset(epst, cfg.EPS)
            ones_fr = ones2d.bitcast(f32r)
            ones1_fr = ones1.bitcast(f32r)

            def load_w(dram_l, name):
                t = wa.tile([P, C, D], bf16, tag="w", name=name)
                nc.sync.dma_start(out=t, in_=dram_l.rearrange("(c p) e -> p c e", p=P))
                return t

            def emit_ln(l, j, k, biasp):
                """x32[:, :, jslice] = LN(x32[:, :, jslice]); also update xbf."""
                js = slice(j * S2, (j + 1) * S2)
                t = x32[:, :, js]
                scratch = sp.tile([P, C, S2], f32, tag="scr", name="xsq")
                nc.vector.tensor_mul(scratch, t, t)
                ps_s = pacc.tile([P, S2], f32, tag="acc", name="ps_s")
                ps_q = pacc.tile([P, S2], f32, tag="acc", name="ps_q")
                for kc in range(C):
                    nc.tensor.matmul(ps_s, ones_fr, t[:, kc, :].bitcast(f32r),
                                     start=(kc == 0), stop=(kc == C - 1))
                for kc in range(C):
                    nc.tensor.matmul(ps_q, ones_fr, scratch[:, kc, :].bitcast(f32r),
                                     start=(kc == 0), stop=(kc == C - 1))
                t1 = st.tile([P, S2], f32, tag="stat", name="t1")
                t2 = st.tile([P, S2], f32, tag="stat", name="t2")
                inv_d = 1.0 / D
                nc.vector.tensor_scalar_mul(t1, ps_s, inv_d)        # mean
                nc.vector.tensor_scalar_mul(t2, ps_q, inv_d)        # E[x^2]
                nc.scalar.activation(ps_s, t1, Square)              # mean^2 -> psum
                nc.vector.tensor_sub(t2, t2, ps_s)                  # var
                nc.scalar.activation(t2, t2, Sqrt, bias=epst)       # sqrt(var+eps)
                nc.vector.reciprocal(t2, t2)                        # rstd
                nc.vector.tensor_mul(t1, t1, t2)                    # mean*rstd
                a_b = t1.tensor is not None  # noqa (clarity)
                rstd_b = t2.unsqueeze(1).to_broadcast((P, C, S2))
                mr_b = t1.unsqueeze(1).to_broadcast((P, C, S2))
                nc.vector.tensor_mul(scratch, t, rstd_b)
                nc.vector.tensor_sub(scratch, scratch, mr_b)        # xnorm
                for kc in range(C):
                    w_ap = biasp[:, OB_LNW + k * C + kc: OB_LNW + k * C + kc + 1]
                    b_ap = biasp[:, OB_LNB + k * C + kc: OB_LNB + k * C + kc + 1]
                    nc.vector.tensor_scalar(x32[:, kc, js], scratch[:, kc, :],
                                            w_ap, b_ap, op0=mult, op1=addop)
                    nc.vector.tensor_scalar(xbf[:, kc, js], scratch[:, kc, :],
                                            w_ap, b_ap, op0=mult, op1=addop)

            def emit_attn(l, is_self, biasp, bvb):
                """One attention sublayer (self w/ causal mask, or cross)."""
                if is_self:
                    dq, dk, dv, do = dwq_s[l], dwk_s[l], dwv_s[l], dwo_s[l]
                    ob_q, ob_k, ob_o, ln_k = OB_Q, OB_K, OB_O, 0
                    kvsrc = xbf
                    bv_idx = 0
                else:
                    dq, dk, dv, do = dwq_c[l], dwk_c[l], dwv_c[l], dwo_c[l]
                    ob_q, ob_k, ob_o, ln_k = OB_CQ, OB_CK, OB_CO, 1
                    kvsrc = memt
                    bv_idx = 1

                wq_t = load_w(dq, "wq_t")
                wk_t = load_w(dk, "wk_t")
                wv_t = load_w(dv, "wv_t")

                # K^T [e, s] for all tokens
                kT = kvp.tile([P, C, S], bf16, tag="kv", name="kT")
                for m in range(C):
                    for n in range(NJ):
                        ps = pacc.tile([P, S2], f32, tag="acc", name="ps_k")
                        for kc in range(C):
                            nc.tensor.matmul(
                                ps, wk_t[:, kc, m * P:(m + 1) * P],
                                kvsrc[:, kc, n * S2:(n + 1) * S2],
                                start=(kc == 0), stop=(kc == C - 1))
                        nc.scalar.activation(
                            kT[:, m, n * S2:(n + 1) * S2], ps, Ident,
                            bias=biasp[:, ob_k + m: ob_k + m + 1])

                # V in natural layout [sk, e] + ones column per head
                HPV = (D // 2) // DH      # heads per v half
                vpad = kvp.tile([P, SK, H, DH + 1], bf16, tag="kv", name="vpad")
                nc.vector.memset(vpad[:, :, :, DH:DH + 1], 1.0)
                for ms in range(SK):
                    for nv in range(2):
                        wide = D // 2
                        ps = pacc.tile([P, S2], f32, tag="acc", name="ps_v")
                        psv = ps[:, :wide]
                        for kc in range(C):
                            nc.tensor.matmul(
                                psv, kvsrc[:, kc, ms * P:(ms + 1) * P],
                                wv_t[:, kc, nv * wide:(nv + 1) * wide],
                                start=(kc == 0), stop=(kc == C - 1))
                        nc.vector.tensor_add(
                            vpad[:, ms, nv * HPV:(nv + 1) * HPV, 0:DH],
                            psv.rearrange("p (h e) -> p h e", e=DH),
                            bvb[:, bv_idx, nv * wide:(nv + 1) * wide]
                               .rearrange("p (h e) -> p h e", e=DH))

                wo_t = load_w(do, "wo_t")

                for j in range(NJ):
                    js = slice(j * S2, (j + 1) * S2)
                    # Q^T for this sq tile (reads CURRENT xbf, pre-LN for j)
                    qT = qp.tile([P, C, S2], bf16, tag="q", name="qT")
                    for m in range(C):
                        ps = pacc.tile([P, S2], f32, tag="acc", name="ps_qp")
                        for kc in range(C):
                            nc.tensor.matmul(ps, wq_t[:, kc, m * P:(m + 1) * P],
                                             xbf[:, kc, js],
                                             start=(kc == 0), stop=(kc == C - 1))
                        nc.scalar.activation(qT[:, m, :], ps, Ident,
                                             bias=biasp[:, ob_q + m: ob_q + m + 1])

                    ctx = cp.tile([P, C, S2], bf16, tag="ctx", name="ctx")
                    nblk = min(SK, (j + 1) * S2 // P) if is_self else SK
                    for h in range(H):
                        hc, hr = h // 2, (h % 2) * DH
                        pav = pavp.tile([DH + 1, S2], f32, tag="av", name="pav")
                        for idx in range(nblk):
                            psc = pfast.tile([P, S2], f32, tag="sc", name="psc")
                            nc.tensor.matmul(
                                psc, kT[hr:hr + DH, hc, idx * P:(idx + 1) * P],
                                qT[hr:hr + DH, hc, :], start=True, stop=True)
                            p_i = pp.tile([P, S2], bf16, tag="p", name="p_i")
                            nc.scalar.activation(p_i, psc, Exp, scale=1.0 / 8.0)
                            mr = idx - j * R
                            if is_self and 0 <= mr < R:
                                nc.vector.tensor_mul(p_i, p_i, masks[:, mr, :])
                            nc.tensor.matmul(pav, vpad[:, idx, h, :], p_i,
                                             start=(idx == 0), stop=(idx == nblk - 1))
                        # normalize by the ones-row sum
                        rsb = st.tile([1, S2], f32, tag="stat", name="rsb")
                        nc.vector.reciprocal(rsb, pav[DH:DH + 1, :])
                        prb = pfast.tile([DH, S2], f32, tag="sc", name="prb")
                        nc.tensor.matmul(prb, ones1_fr, rsb.bitcast(f32r),
                                         start=True, stop=True)
                        rbs = st.tile([DH, S2], f32, tag="stat", name="rbs")
                        nc.scalar.copy(rbs, prb)
                        nc.vector.tensor_mul(ctx[hr:hr + DH, hc, :],
                                             pav[0:DH, :], rbs)

                    # out-proj + bias into scratch, residual into x32, then LN
                    scratch = sp.tile([P, C, S2], f32, tag="scr", name="attno")
                    for m in range(C):
                        ps = pacc.tile([P, S2], f32, tag="acc", name="ps_o")
                        for kc in range(C):
                            nc.tensor.matmul(ps, wo_t[:, kc, m * P:(m + 1) * P],
                                             ctx[:, kc, :],
                                             start=(kc == 0), stop=(kc == C - 1))
                        nc.scalar.activation(scratch[:, m, :], ps, Ident,
                                             bias=biasp[:, ob_o + m: ob_o + m + 1])
                    nc.vector.tensor_add(x32[:, :, js], x32[:, :, js], scratch)
                    emit_ln(l, j, ln_k, biasp)

            def emit_ffn(l, biasp):
                FH = FC // 2
                w1a = wf.tile([P, C, DFF // 2], bf16, tag="wf", name="w1a")
                nc.sync.dma_start(
                    out=w1a,
                    in_=dw1[l].rearrange("(c p) f -> p c f", p=P)[:, :, :DFF // 2])
                w1b = wf.tile([P, C, DFF // 2], bf16, tag="wf", name="w1b")
                nc.sync.dma_start(
                    out=w1b,
                    in_=dw1[l].rearrange("(c p) f -> p c f", p=P)[:, :, DFF // 2:])
                w2v = dw2[l].rearrange("(c p) d -> p c d", p=P)
                w2a = wf.tile([P, FH, D], bf16, tag="wf", name="w2a")
                nc.sync.dma_start(out=w2a, in_=w2v[:, :FH, :])
                w2b = wf.tile([P, FH, D], bf16, tag="wf", name="w2b")
                nc.sync.dma_start(out=w2b, in_=w2v[:, FH:, :])

                for j in range(NJ):
                    js = slice(j * S2, (j + 1) * S2)
                    h_t = hp.tile([P, FC, S2], bf16, tag="h", name="h_t")
                    for fm in range(FC):
                        half, w1x = (0, w1a) if fm < FH else (1, w1b)
                        fcol = fm * P - half * (DFF // 2)
                        ps = pacc.tile([P, S2], f32, tag="acc", name="ps_h")
                        for kc in range(C):
                            nc.tensor.matmul(ps, w1x[:, kc, fcol:fcol + P],
                                             xbf[:, kc, js],
                                             start=(kc == 0), stop=(kc == C - 1))
                        nc.scalar.activation(h_t[:, fm, :], ps, Relu,
                                             bias=biasp[:, OB_B1 + fm: OB_B1 + fm + 1])
                    scratch = sp.tile([P, C, S2], f32, tag="scr", name="ffno")
                    for m in range(C):
                        ps = pacc.tile([P, S2], f32, tag="acc", name="ps_f2")
                        for fc2 in range(FC):
                            w2x = w2a if fc2 < FH else w2b
                            nc.tensor.matmul(ps, w2x[:, fc2 % FH, m * P:(m + 1) * P],
                                             h_t[:, fc2, :],
                                             start=(fc2 == 0), stop=(fc2 == FC - 1))
                        nc.scalar.activation(scratch[:, m, :], ps, Ident,
                                             bias=biasp[:, OB_B2 + m: OB_B2 + m + 1])
                    nc.vector.tensor_add(x32[:, :, js], x32[:, :, js], scratch)
                    emit_ln(l, j, 2, biasp)

            # ================= layer loop =================
            for l in range(L):
                biasp = bpool.tile([P, NB], f32, tag="biasp", name="biasp")
                nc.sync.dma_start(out=biasp, in_=dbias[l])
                bvb = bpool.tile([P, 2, D], bf16, tag="bvb", name="bvb")
                nc.sync.dma_start(out=bvb, in_=dbvf[l].to_broadcast((P, 2, D)))
                emit_attn(l, True, biasp, bvb)
                emit_attn(l, False, biasp, bvb)
                emit_ffn(l, biasp)

            # ================= final projection =================
            wp_t = wa.tile([P, C, D], bf16, tag="w", name="wp_t")
            nc.sync.dma_start(out=wp_t, in_=dwp.rearrange("(c p) e -> p c e", p=P))
            for m in range(C):
                for j in range(NJ):
                    ps = pacc.tile([P, S2], f32, tag="acc", name="ps_p")
                    for kc in range(C):
                        nc.tensor.matmul(ps, wp_t[:, kc, m * P:(m + 1) * P],
                                         xbf[:, kc, j * S2:(j + 1) * S2],
                                         start=(kc == 0), stop=(kc == C - 1))
                    o_sb = st.tile([P, S2], f32, tag="stat", name="o_sb")
                    nc.scalar.activation(o_sb, ps, Ident,
                                         bias=bp_sb[:, m:m + 1])
                    nc.sync.dma_start(out=doutv[:, m, j * S2:(j + 1) * S2], in_=o_sb)

    return nc


# ---------------- host-side prep + run ----------------------------------------
def prepare_inputs(cfg, inputs):
    """Build the per-core in_maps from full reference inputs."""
    P, C, S, D, H, DH, L, DFF = (cfg.P, cfg.C, cfg.S, cfg.D, cfg.H, cfg.DH,
                                 cfg.L, cfg.DFF)
    FC, S2, R = cfg.FC, cfg.S2, cfg.R
    f32 = np.float32

    ep = np.asarray(inputs["encoded_patches"], dtype=f32)       # [B, S, D]
    pe = _sinusoidal_pe(S, D)
    x0 = ep + pe[None]

    def bt(a):
        return np.ascontiguousarray(np.asarray(a, dtype=f32).astype(BF16))

    shared = {}
    for nm, w in (("wq_s", "self_in_w"), ("wq_c", "cross_in_w")):
        iw = np.asarray(inputs[w], dtype=f32)                   # [L, 3D, D]
        pre = nm[-1]
        shared[f"wq_{pre}"] = bt(iw[:, :D, :].transpose(0, 2, 1))
        shared[f"wk_{pre}"] = bt(iw[:, D:2 * D, :].transpose(0, 2, 1))
        shared[f"wv_{pre}"] = bt(iw[:, 2 * D:, :].transpose(0, 2, 1))
    shared["wo_s"] = bt(np.asarray(inputs["self_out_w"], dtype=f32).transpose(0, 2, 1))
    shared["wo_c"] = bt(np.asarray(inputs["cross_out_w"], dtype=f32).transpose(0, 2, 1))
    shared["w1"] = bt(np.asarray(inputs["ffn_w1"], dtype=f32).transpose(0, 2, 1))
    shared["w2"] = bt(np.asarray(inputs["ffn_w2"], dtype=f32).transpose(0, 2, 1))
    shared["wp"] = bt(np.asarray(inputs["to_patch_w"], dtype=f32).T)

    def cols(v, nch):
        return np.asarray(v, dtype=f32).reshape(nch, P).T       # [P, nch]

    NB = 13 * C + FC
    biasp = np.zeros((L, P, NB), dtype=f32)
    sib = np.asarray(inputs["self_in_b"], dtype=f32)
    cib = np.asarray(inputs["cross_in_b"], dtype=f32)
    sob = np.asarray(inputs["self_out_b"], dtype=f32)
    cob = np.asarray(inputs["cross_out_b"], dtype=f32)
    b1 = np.asarray(inputs["ffn_b1"], dtype=f32)
    b2 = np.asarray(inputs["ffn_b2"], dtype=f32)
    lnw = np.asarray(inputs["ln_w"], dtype=f32)
    lnb = np.asarray(inputs["ln_b"], dtype=f32)
    for l in range(L):
        biasp[l, :, 0:C] = cols(sib[l][:D], C)
        biasp[l, :, C:2 * C] = cols(sib[l][D:2 * D], C)
        biasp[l, :, 2 * C:3 * C] = cols(sob[l], C)
        biasp[l, :, 3 * C:4 * C] = cols(cib[l][:D], C)
        biasp[l, :, 4 * C:5 * C] = cols(cib[l][D:2 * D], C)
        biasp[l, :, 5 * C:6 * C] = cols(cob[l], C)
        biasp[l, :, 6 * C:6 * C + FC] = cols(b1[l], FC)
        biasp[l, :, 6 * C + FC:7 * C + FC] = cols(b2[l], C)
        for k in range(3):
            biasp[l, :, 7 * C + FC + k * C:7 * C + FC + (k + 1) * C] = cols(lnw[l, k], C)
            biasp[l, :, 10 * C + FC + k * C:10 * C + FC + (k + 1) * C] = cols(lnb[l, k], C)
    shared["biasp"] = biasp
    bvf = np.stack([sib[:, 2 * D:], cib[:, 2 * D:]], axis=1)    # [L, 2, D]
    shared["bvf"] = np.ascontiguousarray(bvf.astype(BF16))
    shared["bp6"] = cols(np.asarray(inputs["to_patch_b"], dtype=f32), C)

    m = np.zeros((R, P, S2), dtype=f32)
    for r in range(R):
        pidx = np.arange(P)[:, None]
        fidx = np.arange(S2)[None, :]
        m[r] = ((r * P + pidx) <= fidx).astype(f32)
    shared["masks"] = np.ascontiguousarray(m.astype(BF16))

    in_maps = []
    for b in range(cfg.n_cores):
        im = dict(shared)
        xt = np.ascontiguousarray(x0[b].T)                      # [D, S]
        im["x0t32"] = xt
        im["x0tbf"] = np.ascontiguousarray(xt.astype(BF16))
        im["memtbf"] = np.ascontiguousarray(ep[b].T.astype(BF16))
        in_maps.append(im)
    return in_maps


_NC_CACHE = {}


def run(inputs, cfg=FULL, trace=False):
    """Returns (patches [B, S, D] float32, exec_time_ns or None)."""
    from concourse.bass_utils import run_bass_kernel_spmd

    key = (cfg.B, cfg.S, cfg.D, cfg.H, cfg.L, cfg.DFF)
    if key not in _NC_CACHE:
        _NC_CACHE[key] = build_nc(cfg)
    nc = _NC_CACHE[key]
    in_maps = prepare_inputs(cfg, inputs)
    res = run_bass_kernel_spmd(nc, in_maps, core_ids=list(range(cfg.n_cores)),
                               trace=trace)
    patches = np.stack([res.results[b]["outt"].T for b in range(cfg.n_cores)])
    return patches.astype(np.float32), res.exec_time_ns


def kernel(**inputs):
    cfg = FULL
    patches, _ = run(inputs, cfg)                               # [B, S, D]
    B = cfg.B
    img = int(np.sqrt(cfg.S)) * 16                              # 512
    out = patches.reshape(B, img, img, 3).transpose(0, 3, 1, 2)
    return np.ascontiguousarray(out)


# revision 17
# speedup vs baseline: 1.0217x; 1.0217x over previous
"""Trainium2 Bass kernel for a 6-layer transformer decoder (post-norm, ViT-style).

Sharding: data-parallel over batch — 8 cores x 1 batch element, no collectives.
All activations are kept TRANSPOSED in SBUF ([feature, token]) so every linear
layer is a plain lhsT.T @ rhs matmul with no transposes anywhere.

Softmax is computed over the PARTITION dim (scores stored [sk, sq]); no max
subtraction is needed (scores are O(1) after layernorm), and the denominator
comes for free from a ones-column appended to V in the attn@V matmul.
LayerNorm reductions over the feature dim (= partitions) use all-ones [128,128]
matmuls in float32r, which replicate the sums across partitions for free.
"""

import numpy as np
import ml_dtypes

BF16 = ml_dtypes.bfloat16


class Cfg:
    def __init__(self, B=8, S=1024, D=768, H=12, L=6, DFF=2048, n_cores=8):
        self.B, self.S, self.D, self.H, self.L, self.DFF = B, S, D, H, L, DFF
        self.n_cores = n_cores
        self.P = 128
        self.DH = 64                      # head dim (fixed by reference)
        assert D % self.P == 0 and D // H == self.DH
        self.C = D // self.P              # d-chunks
        self.S2 = min(512, S)             # sq tile (matmul N)
        assert S % self.S2 == 0
        self.NJ = S // self.S2            # sq tiles
        self.SK = S // self.P             # sk chunks
        self.FC = DFF // self.P           # ffn f-chunks
        assert self.FC % 4 == 0
        self.R = self.S2 // self.P        # diag-mask variants
        self.EPS = 1e-5


FULL = Cfg()


def _sinusoidal_pe(max_len, d):
    pos = np.arange(max_len)[:, None].astype(np.float32)
    div = np.exp(np.arange(0, d, 2).astype(np.float32) * (-np.log(10000.0) / d))
    pe = np.zeros((max_len, d), dtype=np.float32)
    pe[:, 0::2] = np.sin(pos * div)
    pe[:, 1::2] = np.cos(pos * div)
    return pe


# ---------------- bass module builder ----------------------------------------
def build_nc(cfg):
    import concourse.bass as bass  # noqa: F401
    import concourse.bacc as bacc
    import concourse.mybir as mybir
    import concourse.tile as tile

    P, C, S, S2, NJ, SK, FC, H, DH, L, D, DFF = (
        cfg.P, cfg.C, cfg.S, cfg.S2, cfg.NJ, cfg.SK, cfg.FC, cfg.H, cfg.DH,
        cfg.L, cfg.D, cfg.DFF)
    R = cfg.R
    W = (R - 1) * P                       # causal mask extended-tile offset
    f32 = mybir.dt.float32
    f32r = mybir.dt.float32r
    bf16 = mybir.dt.bfloat16
    Ident = mybir.ActivationFunctionType.Identity
    Exp = mybir.ActivationFunctionType.Exp
    Relu = mybir.ActivationFunctionType.Relu
    Sqrt = mybir.ActivationFunctionType.Sqrt
    Square = mybir.ActivationFunctionType.Square
    mult = mybir.AluOpType.mult
    addop = mybir.AluOpType.add
    maxop = mybir.AluOpType.max

    NB = 13 * C + FC
    OB_Q, OB_K, OB_O = 0, C, 2 * C
    OB_CQ, OB_CK, OB_CO = 3 * C, 4 * C, 5 * C
    OB_B1 = 6 * C
    OB_B2 = 6 * C + FC
    OB_LNW = 7 * C + FC
    OB_LNB = 10 * C + FC

    nc = bacc.Bacc("TRN2", name="decoder")

    dx32 = nc.dram_tensor("x0t32", [D, S], f32, kind="ExternalInput")[:]
    dxbf = nc.dram_tensor("x0tbf", [D, S], bf16, kind="ExternalInput")[:]
    dmem = nc.dram_tensor("memtbf", [D, S], bf16, kind="ExternalInput")[:]
    dwq_s = nc.dram_tensor("wq_s", [L, D, D], bf16, kind="ExternalInput")[:]
    dwk_s = nc.dram_tensor("wk_s", [L, D, D], bf16, kind="ExternalInput")[:]
    dwv_s = nc.dram_tensor("wv_s", [L, D, D], bf16, kind="ExternalInput")[:]
    dwo_s = nc.dram_tensor("wo_s", [L, D, D], bf16, kind="ExternalInput")[:]
    dwq_c = nc.dram_tensor("wq_c", [L, D, D], bf16, kind="ExternalInput")[:]
    dwk_c = nc.dram_tensor("wk_c", [L, D, D], bf16, kind="ExternalInput")[:]
    dwv_c = nc.dram_tensor("wv_c", [L, D, D], bf16, kind="ExternalInput")[:]
    dwo_c = nc.dram_tensor("wo_c", [L, D, D], bf16, kind="ExternalInput")[:]
    dw1 = nc.dram_tensor("w1", [L, D, DFF], bf16, kind="ExternalInput")[:]
    dw2 = nc.dram_tensor("w2", [L, DFF, D], bf16, kind="ExternalInput")[:]
    dwp = nc.dram_tensor("wp", [D, D], bf16, kind="ExternalInput")[:]
    dbias = nc.dram_tensor("biasp", [L, P, NB], f32, kind="ExternalInput")[:]
    dbvf = nc.dram_tensor("bvf", [L, 2, D], bf16, kind="ExternalInput")[:]
    dbp = nc.dram_tensor("bp6", [P, C], f32, kind="ExternalInput")[:]
    dmask = nc.dram_tensor("maske", [P, W + S2], bf16, kind="ExternalInput")[:]
    dout = nc.dram_tensor("outt", [D, S], f32, kind="ExternalOutput")[:]

    dx32v = dx32.rearrange("(c p) s -> p c s", p=P)
    dxbfv = dxbf.rearrange("(c p) s -> p c s", p=P)
    dmemv = dmem.rearrange("(c p) s -> p c s", p=P)
    doutv = dout.rearrange("(c p) s -> p c s", p=P)

    with tile.TileContext(nc) as tc:
        with (
            tc.tile_pool(name="singles", bufs=1) as singles,
            tc.tile_pool(name="wa", bufs=2) as wa,
            tc.tile_pool(name="wf", bufs=2) as wf,
            tc.tile_pool(name="bp", bufs=1) as bpool,
            tc.tile_pool(name="kv", bufs=2) as kvp,
            tc.tile_pool(name="qp", bufs=1) as qp,
            tc.tile_pool(name="cp", bufs=1) as cp,
            tc.tile_pool(name="pp", bufs=3) as pp,
            tc.tile_pool(name="hp", bufs=1) as hp,
            tc.tile_pool(name="sp", bufs=1) as sp,
            tc.tile_pool(name="st", bufs=2) as st,
            tc.tile_pool(name="pacc", bufs=2, space="PSUM") as pacc,
            tc.tile_pool(name="pfast", bufs=3, space="PSUM") as pfast,
            tc.tile_pool(name="pav", bufs=2, space="PSUM") as pavp,
        ):
            # ---- static tiles ----
            x32 = singles.tile([P, C, S], f32, tag="x32")
            xbf = singles.tile([P, C, S], bf16, tag="xbf")
            memt = singles.tile([P, C, S], bf16, tag="memt")
            maske = singles.tile([P, W + S2], bf16, tag="maske")
            ones2b = singles.tile([P, P], bf16, tag="ones2b")
            epst = singles.tile([P, 1], f32, tag="epst")
            bp_sb = singles.tile([P, C], f32, tag="bp_sb")

            nc.sync.dma_start(out=x32, in_=dx32v)
            nc.sync.dma_start(out=xbf, in_=dxbfv)
            nc.sync.dma_start(out=memt, in_=dmemv)
            nc.sync.dma_start(out=maske, in_=dmask)
            nc.sync.dma_start(out=bp_sb, in_=dbp)
            nc.vector.memset(ones2b, 1.0)
            nc.vector.memset(epst, cfg.EPS)

            def load_w(dram_l, name):
                t = wa.tile([P, C, D], bf16, tag="w", name=name)
                nc.sync.dma_start(out=t, in_=dram_l.rearrange("(c p) e -> p c e", p=P))
                return t

            def emit_ln(j, k, biasp):
                """x32[:, :, js] = LN(x32[:, :, js]) * w + b; update xbf too."""
                js = slice(j * S2, (j + 1) * S2)
                t = x32[:, :, js]
                scratch = sp.tile([P, C, S2], f32, tag="scr", name="lnscr")
                ps_s = pacc.tile([P, S2], f32, tag="acc", name="ps_s")
                ps_q = pacc.tile([P, S2], f32, tag="acc", name="ps_q")
                for kc in range(C):
                    tb = pp.tile([P, S2], bf16, tag="p", name="tb")
                    nc.vector.tensor_copy(tb, t[:, kc, :])
                    xq = pp.tile([P, S2], bf16, tag="p", name="xq")
                    nc.vector.tensor_mul(xq, tb, tb)
                    nc.tensor.matmul(ps_s, ones2b, tb,
                                     start=(kc == 0), stop=(kc == C - 1))
                    nc.tensor.matmul(ps_q, ones2b, xq,
                                     start=(kc == 0), stop=(kc == C - 1))
                t1 = st.tile([P, S2], f32, tag="stat", name="t1")
                t2 = st.tile([P, S2], f32, tag="stat", name="t2")
                inv_d = 1.0 / D
                nc.vector.tensor_scalar_mul(t1, ps_s, inv_d)     # mean
                nc.vector.tensor_scalar_mul(t2, ps_q, inv_d)     # E[x^2]
                nc.scalar.activation(ps_s, t1, Square)           # mean^2 -> psum
                nc.vector.tensor_sub(t2, t2, ps_s)               # var
                nc.scalar.activation(t2, t2, Sqrt, bias=epst)    # sqrt(var+eps)
                nc.vector.reciprocal(t2, t2)                     # rstd
                nc.vector.tensor_mul(t1, t1, t2)                 # mean*rstd
                rstd_b = t2.unsqueeze(1).to_broadcast((P, C, S2))
                mr_b = t1.unsqueeze(1).to_broadcast((P, C, S2))
                nc.vector.tensor_mul(scratch, t, rstd_b)
                nc.vector.tensor_sub(scratch, scratch, mr_b)     # xnorm
                for kc in range(C):
                    w_ap = biasp[:, OB_LNW + k * C + kc: OB_LNW + k * C + kc + 1]
                    b_ap = biasp[:, OB_LNB + k * C + kc: OB_LNB + k * C + kc + 1]
                    nc.vector.tensor_scalar(x32[:, kc, js], scratch[:, kc, :],
                                            w_ap, b_ap, op0=mult, op1=addop)
                    nc.vector.tensor_scalar(xbf[:, kc, js], scratch[:, kc, :],
                                            w_ap, b_ap, op0=mult, op1=addop)

            def emit_attn(l, is_self, biasp, bvb):
                if is_self:
                    dq, dk, dv, do = dwq_s[l], dwk_s[l], dwv_s[l], dwo_s[l]
                    ob_q, ob_k, ob_o, ln_k = OB_Q, OB_K, OB_O, 0
                    kvsrc, bv_idx = xbf, 0
                else:
                    dq, dk, dv, do = dwq_c[l], dwk_c[l], dwv_c[l], dwo_c[l]
                    ob_q, ob_k, ob_o, ln_k = OB_CQ, OB_CK, OB_CO, 1
                    kvsrc, bv_idx = memt, 1

                wk_t = load_w(dk, "wk_t")
                wv_t = load_w(dv, "wv_t")

                # K^T [e, s] for all tokens
                kT = kvp.tile([P, C, S], bf16, tag="kv", name="kT")
                for m in range(C):
                    for n in range(NJ):
                        ps = pacc.tile([P, S2], f32, tag="acc", name="ps_k")
                        for kc in range(C):
                            nc.tensor.matmul(
                                ps, wk_t[:, kc, m * P:(m + 1) * P],
                                kvsrc[:, kc, n * S2:(n + 1) * S2],
                                start=(kc == 0), stop=(kc == C - 1))
                        nc.vector.tensor_scalar_add(
                            kT[:, m, n * S2:(n + 1) * S2], ps,
                            biasp[:, ob_k + m: ob_k + m + 1])

                # V in natural layout [sk, e] + ones column per head
                HPV = (D // 2) // DH      # heads per v half
                wide = D // 2
                vpad = kvp.tile([P, SK, H, DH + 1], bf16, tag="kv", name="vpad")
                nc.vector.memset(vpad[:, :, :, DH:DH + 1], 1.0)
                for ms in range(SK):
                    for nv in range(2):
                        ps = pacc.tile([P, S2], f32, tag="acc", name="ps_v")
                        psv = ps[:, :wide]
                        for kc in range(C):
                            nc.tensor.matmul(
                                psv, kvsrc[:, kc, ms * P:(ms + 1) * P],
                                wv_t[:, kc, nv * wide:(nv + 1) * wide],
                                start=(kc == 0), stop=(kc == C - 1))
                        nc.vector.tensor_add(
                            vpad[:, ms, nv * HPV:(nv + 1) * HPV, 0:DH],
                            psv.rearrange("p (h e) -> p h e", e=DH),
                            bvb[:, bv_idx, nv * wide:(nv + 1) * wide]
                               .rearrange("p (h e) -> p h e", e=DH))

                wq_t = load_w(dq, "wq_t")
                wo_t = load_w(do, "wo_t")

                for j in range(NJ):
                    js = slice(j * S2, (j + 1) * S2)
                    qT = qp.tile([P, C, S2], bf16, tag="q", name="qT")
                    for m in range(C):
                        ps = pacc.tile([P, S2], f32, tag="acc", name="ps_qp")
                        for kc in range(C):
                            nc.tensor.matmul(ps, wq_t[:, kc, m * P:(m + 1) * P],
                                             xbf[:, kc, js],
                                             start=(kc == 0), stop=(kc == C - 1))
                        nc.vector.tensor_scalar_add(
                            qT[:, m, :], ps, biasp[:, ob_q + m: ob_q + m + 1])

                    ctx = cp.tile([P, C, S2], bf16, tag="ctx", name="ctx")
                    nblk = min(SK, (j + 1) * S2 // P) if is_self else SK
                    for h in range(H):
                        hc, hr = h // 2, (h % 2) * DH
                        pav = pavp.tile([DH + 1, S2], f32, tag="av", name="pav")
                        for idx in range(nblk):
                            psc = pfast.tile([P, S2], f32, tag="sc", name="psc")
                            nc.tensor.matmul(
                                psc, kT[hr:hr + DH, hc, idx * P:(idx + 1) * P],
                                qT[hr:hr + DH, hc, :], start=True, stop=True)
                            p_i = pp.tile([P, S2], bf16, tag="p", name="p_i")
                            nc.scalar.activation(p_i, psc, Exp, scale=1.0 / 8.0)
                            mr = idx - j * R
                            if is_self and 0 <= mr < R:
                                nc.vector.tensor_mul(
                                    p_i, p_i, maske[:, W - mr * P: 2 * W - mr * P + S2 - W])
                            nc.tensor.matmul(pav, vpad[:, idx, h, :], p_i,
                                             start=(idx == 0), stop=(idx == nblk - 1))
                        rsb = st.tile([1, S2], f32, tag="rsb", name="rsb")
                        nc.vector.reciprocal(rsb, pav[DH:DH + 1, :])
                        rbs = st.tile([DH, S2], f32, tag="stat", name="rbs")
                        nc.gpsimd.partition_broadcast(rbs, rsb, channels=DH)
                        nc.vector.tensor_mul(ctx[hr:hr + DH, hc, :],
                                             pav[0:DH, :], rbs)

                    for m in range(C):
                        ps = pacc.tile([P, S2], f32, tag="acc", name="ps_o")
                        for kc in range(C):
                            nc.tensor.matmul(ps, wo_t[:, kc, m * P:(m + 1) * P],
                                             ctx[:, kc, :],
                                             start=(kc == 0), stop=(kc == C - 1))
                        nc.vector.scalar_tensor_tensor(
                            x32[:, m, js], ps, biasp[:, ob_o + m: ob_o + m + 1],
                            x32[:, m, js], op0=addop, op1=addop)
                    emit_ln(j, ln_k, biasp)

            def emit_ffn(l, biasp):
                FH = FC // 2
                FQ = FC // 4
                w2v = dw2[l].rearrange("(c p) d -> p c d", p=P)
                w2a = wf.tile([P, FH, D], bf16, tag="wf2", name="w2a")
                nc.sync.dma_start(out=w2a, in_=w2v[:, :FH, :])
                w2b = wf.tile([P, FH, D], bf16, tag="wf2", name="w2b")
                nc.sync.dma_start(out=w2b, in_=w2v[:, FH:, :])
                w1v = dw1[l].rearrange("(c p) f -> p c f", p=P)

                for j in range(NJ):
                    js = slice(j * S2, (j + 1) * S2)
                    h_t = hp.tile([P, FC, S2], bf16, tag="h", name="h_t")
                    for q in range(4):                    # w1 quarter-streamed
                        w1x = wf.tile([P, C, FQ * P], bf16, tag="wf1", name="w1x")
                        nc.sync.dma_start(
                            out=w1x, in_=w1v[:, :, q * FQ * P:(q + 1) * FQ * P])
                        for fi in range(FQ):
                            fm = q * FQ + fi
                            ps = pacc.tile([P, S2], f32, tag="acc", name="ps_h")
                            for kc in range(C):
                                nc.tensor.matmul(ps, w1x[:, kc, fi * P:(fi + 1) * P],
                                                 xbf[:, kc, js],
                                                 start=(kc == 0), stop=(kc == C - 1))
                            nc.vector.tensor_scalar(
                                h_t[:, fm, :], ps,
                                biasp[:, OB_B1 + fm: OB_B1 + fm + 1], 0.0,
                                op0=addop, op1=maxop)
                    for m in range(C):
                        ps = pacc.tile([P, S2], f32, tag="acc", name="ps_f2")
                        for fc2 in range(FC):
                            w2x = w2a if fc2 < FH else w2b
                            nc.tensor.matmul(ps, w2x[:, fc2 % FH, m * P:(m + 1) * P],
                                             h_t[:, fc2, :],
                                             start=(fc2 == 0), stop=(fc2 == FC - 1))
                        nc.vector.scalar_tensor_tensor(
                            x32[:, m, js], ps, biasp[:, OB_B2 + m: OB_B2 + m + 1],
                            x32[:, m, js], op0=addop, op1=addop)
                    emit_ln(j, 2, biasp)

            # ================= layer loop =================
            for l in range(L):
                biasp = bpool.tile([P, NB], f32, tag="biasp", name="biasp", bufs=2)
                nc.sync.dma_start(out=biasp, in_=dbias[l])
                bvb = bpool.tile([P, 2, D], bf16, tag="bvb", name="bvb", bufs=1)
                nc.sync.dma_start(out=bvb, in_=dbvf[l].partition_broadcast(P))
                emit_attn(l, True, biasp, bvb)
                emit_attn(l, False, biasp, bvb)
                emit_ffn(l, biasp)

            # ================= final projection =================
            wp_t = wa.tile([P, C, D], bf16, tag="w", name="wp_t")
            nc.sync.dma_start(out=wp_t, in_=dwp.rearrange("(c p) e -> p c e", p=P))
            for m in range(C):
                for j in range(NJ):
                    ps = pacc.tile([P, S2], f32, tag="acc", name="ps_p")
                    for kc in range(C):
                        nc.tensor.matmul(ps, wp_t[:, kc, m * P:(m + 1) * P],
                                         xbf[:, kc, j * S2:(j + 1) * S2],
                                         start=(kc == 0), stop=(kc == C - 1))
                    o_sb = st.tile([P, S2], f32, tag="stat", name="o_sb")
                    nc.scalar.activation(o_sb, ps, Ident, bias=bp_sb[:, m:m + 1])
                    nc.sync.dma_start(out=doutv[:, m, j * S2:(j + 1) * S2], in_=o_sb)

    nc.finalize()
    return nc


# ---------------- host-side prep + run ----------------------------------------
def prepare_inputs(cfg, inputs):
    P, C, S, D, H, DH, L, DFF = (cfg.P, cfg.C, cfg.S, cfg.D, cfg.H, cfg.DH,
                                 cfg.L, cfg.DFF)
    FC, S2, R = cfg.FC, cfg.S2, cfg.R
    W = (R - 1) * P
    f32 = np.float32

    ep = np.asarray(inputs["encoded_patches"], dtype=f32)       # [B, S, D]
    pe = _sinusoidal_pe(S, D)
    x0 = ep + pe[None]

    def bt(a):
        return np.ascontiguousarray(np.asarray(a, dtype=f32).astype(BF16))

    shared = {}
    for pre, w in (("s", "self_in_w"), ("c", "cross_in_w")):
        iw = np.asarray(inputs[w], dtype=f32)                   # [L, 3D, D]
        shared[f"wq_{pre}"] = bt(iw[:, :D, :].transpose(0, 2, 1))
        shared[f"wk_{pre}"] = bt(iw[:, D:2 * D, :].transpose(0, 2, 1))
        shared[f"wv_{pre}"] = bt(iw[:, 2 * D:, :].transpose(0, 2, 1))
    shared["wo_s"] = bt(np.asarray(inputs["self_out_w"], dtype=f32).transpose(0, 2, 1))
    shared["wo_c"] = bt(np.asarray(inputs["cross_out_w"], dtype=f32).transpose(0, 2, 1))
    shared["w1"] = bt(np.asarray(inputs["ffn_w1"], dtype=f32).transpose(0, 2, 1))
    shared["w2"] = bt(np.asarray(inputs["ffn_w2"], dtype=f32).transpose(0, 2, 1))
    shared["wp"] = bt(np.asarray(inputs["to_patch_w"], dtype=f32).T)

    def cols(v, nch):
        return np.asarray(v, dtype=f32).reshape(nch, P).T       # [P, nch]

    NB = 13 * C + FC
    biasp = np.zeros((L, P, NB), dtype=f32)
    sib = np.asarray(inputs["self_in_b"], dtype=f32)
    cib = np.asarray(inputs["cross_in_b"], dtype=f32)
    sob = np.asarray(inputs["self_out_b"], dtype=f32)
    cob = np.asarray(inputs["cross_out_b"], dtype=f32)
    b1 = np.asarray(inputs["ffn_b1"], dtype=f32)
    b2 = np.asarray(inputs["ffn_b2"], dtype=f32)
    lnw = np.asarray(inputs["ln_w"], dtype=f32)
    lnb = np.asarray(inputs["ln_b"], dtype=f32)
    for l in range(L):
        biasp[l, :, 0:C] = cols(sib[l][:D], C)
        biasp[l, :, C:2 * C] = cols(sib[l][D:2 * D], C)
        biasp[l, :, 2 * C:3 * C] = cols(sob[l], C)
        biasp[l, :, 3 * C:4 * C] = cols(cib[l][:D], C)
        biasp[l, :, 4 * C:5 * C] = cols(cib[l][D:2 * D], C)
        biasp[l, :, 5 * C:6 * C] = cols(cob[l], C)
        biasp[l, :, 6 * C:6 * C + FC] = cols(b1[l], FC)
        biasp[l, :, 6 * C + FC:7 * C + FC] = cols(b2[l], C)
        for k in range(3):
            biasp[l, :, 7 * C + FC + k * C:7 * C + FC + (k + 1) * C] = cols(lnw[l, k], C)
            biasp[l, :, 10 * C + FC + k * C:10 * C + FC + (k + 1) * C] = cols(lnb[l, k], C)
    shared["biasp"] = biasp
    bvf = np.stack([sib[:, 2 * D:], cib[:, 2 * D:]], axis=1)    # [L, 2, D]
    shared["bvf"] = np.ascontiguousarray(bvf.astype(BF16))
    shared["bp6"] = cols(np.asarray(inputs["to_patch_b"], dtype=f32), C)

    # extended causal mask: maske[p, g] = 1 iff g - W >= p
    pidx = np.arange(P)[:, None]
    gidx = np.arange(W + S2)[None, :]
    shared["maske"] = np.ascontiguousarray(
        ((gidx - W) >= pidx).astype(f32).astype(BF16))

    in_maps = []
    for b in range(cfg.n_cores):
        im = dict(shared)
        xt = np.ascontiguousarray(x0[b].T)                      # [D, S]
        im["x0t32"] = xt
        im["x0tbf"] = np.ascontiguousarray(xt.astype(BF16))
        im["memtbf"] = np.ascontiguousarray(ep[b].T.astype(BF16))
        in_maps.append(im)
    return in_maps


_NC_CACHE = {}


def run(inputs, cfg=FULL, trace=False):
    """Returns (patches [B, S, D] float32, exec_time_ns or None)."""
    from concourse.bass_utils import run_bass_kernel_spmd

    key = (cfg.B, cfg.S, cfg.D, cfg.H, cfg.L, cfg.DFF, cfg.n_cores)
    if key not in _NC_CACHE:
        _NC_CACHE[key] = build_nc(cfg)
    nc = _NC_CACHE[key]
    in_maps = prepare_inputs(cfg, inputs)
    res = run_bass_kernel_spmd(nc, in_maps, core_ids=list(range(cfg.n_cores)),
                               trace=trace)
    global LAST_RESULT
    LAST_RESULT = res
    patches = np.stack([np.asarray(res.results[b]["outt"], dtype=np.float32).T
                        for b in range(cfg.n_cores)])
    return patches, res.exec_time_ns


def kernel(**inputs):
    cfg = FULL
    patches, _ = run(inputs, cfg)                               # [B, S, D]
    B = cfg.B
    img = 512
    out = patches.reshape(B, img, img, 3).transpose(0, 3, 1, 2)
    return np.ascontiguousarray(out)
